# revision 7
# baseline (speedup 1.0000x reference)
"""Trainium2 Bass kernel: causal depthwise short conv1d + SiLU.

Problem: x [B=4, T=4096, C=2048] f32, kernel [K=4, C=2048] f32.
  y[b, t, c] = silu(sum_j kernel[j, c] * x[b, t - j, c])   (zero left-pad)
  next_cache = x[:, T-K+1:, :]

Strategy:
  - Each (b, c) pair is an independent length-T sequence -> B*C = 8192 rows.
  - Shard channels across the 8 cores: core i handles channels
    [i*256, (i+1)*256) -> 1024 rows of [PAD + T] (zero pre-padded).
  - On-chip layout: partition = row, free dim = time. The 4 taps become
    per-partition tensor_scalar / scalar_tensor_tensor fused mul-adds on
    DVE/GPSIMD; SiLU runs on the scalar engine.
"""

import os
import sys

import numpy as np

_TRN_REPO = "/opt/trn_rl_repo"
if _TRN_REPO not in sys.path:
    sys.path.insert(0, _TRN_REPO)

B, T, C, K = 4, 4096, 2048, 4
PAD = K - 1
NCORES = 8
CPC = C // NCORES       # 256 channels per core
ROWS = CPC * B          # 1024 rows per core
TC = 2048               # time-chunk per tile

_PROG_CACHE: dict = {}


def _build_program(use_silu: bool = True, variant: str = "split", loop: int = 1):
    """Build the per-core Bass program (identical on all 8 cores).

    loop > 1 wraps the whole body in a hardware For_i that redoes the
    (idempotent) work `loop` times — benchmarking only.
    """
    key = (use_silu, variant, loop)
    if key in _PROG_CACHE:
        return _PROG_CACHE[key]

    import concourse.tile as tile
    from concourse import bacc, mybir

    nc = bacc.Bacc("TRN2")
    xs = nc.dram_tensor("xs", [ROWS, PAD + T], mybir.dt.float32, kind="ExternalInput")
    ks = nc.dram_tensor("ks", [ROWS, K], mybir.dt.float32, kind="ExternalInput")
    ys = nc.dram_tensor("ys", [ROWS, T], mybir.dt.float32, kind="ExternalOutput")

    MULT = mybir.AluOpType.mult
    ADD = mybir.AluOpType.add
    F32 = mybir.dt.float32

    xs_ap, ks_ap, ys_ap = xs.ap(), ks.ap(), ys.ap()

    from contextlib import ExitStack

    with tile.TileContext(nc) as tc:
        with ExitStack() as stack:
            kpool = stack.enter_context(tc.tile_pool(name="kp", bufs=2))
            xpool = stack.enter_context(tc.tile_pool(name="xp", bufs=3))
            tpool = stack.enter_context(tc.tile_pool(name="tp", bufs=3))
            ypool = stack.enter_context(tc.tile_pool(name="yp", bufs=3))
            if loop > 1:
                stack.enter_context(tc.For_i(0, loop, 1))
            for r in range(ROWS // 128):
                rows = slice(r * 128, (r + 1) * 128)
                k_sb = kpool.tile([128, K], F32)
                nc.sync.dma_start(k_sb[:], ks_ap[rows, :])
                for it in range(T // TC):
                    x_sb = xpool.tile([128, TC + PAD], F32)
                    nc.sync.dma_start(
                        x_sb[:], xs_ap[rows, it * TC : it * TC + TC + PAD]
                    )
                    # z = sum_j k[:, j] * x[t - j]; x_sb col (PAD + t) = x[t]
                    t0 = tpool.tile([128, TC], F32, tag="t0")
                    nc.vector.tensor_scalar_mul(
                        t0[:], x_sb[:, PAD : PAD + TC], k_sb[:, 0:1]
                    )
                    t1 = tpool.tile([128, TC], F32, tag="t1")
                    nc.vector.scalar_tensor_tensor(
                        t1[:], x_sb[:, PAD - 1 : PAD - 1 + TC], k_sb[:, 1:2],
                        t0[:], MULT, ADD,
                    )
                    t2 = tpool.tile([128, TC], F32, tag="t2")
                    if variant == "split":
                        # tap 2 on GPSIMD to offload the vector engine
                        nc.gpsimd.scalar_tensor_tensor(
                            t2[:], x_sb[:, PAD - 2 : PAD - 2 + TC], k_sb[:, 2:3],
                            t1[:], MULT, ADD,
                        )
                    else:
                        nc.vector.scalar_tensor_tensor(
                            t2[:], x_sb[:, PAD - 2 : PAD - 2 + TC], k_sb[:, 2:3],
                            t1[:], MULT, ADD,
                        )
                    t3 = tpool.tile([128, TC], F32, tag="t3")
                    nc.vector.scalar_tensor_tensor(
                        t3[:], x_sb[:, PAD - 3 : PAD - 3 + TC], k_sb[:, 3:4],
                        t2[:], MULT, ADD,
                    )
                    y_sb = ypool.tile([128, TC], F32)
                    if use_silu:
                        nc.scalar.activation(
                            y_sb[:], t3[:], mybir.ActivationFunctionType.Silu
                        )
                    else:
                        # CoreSim does not implement Silu; emulate as z*sigmoid(z)
                        sg = tpool.tile([128, TC], F32, tag="sg")
                        nc.scalar.activation(
                            sg[:], t3[:], mybir.ActivationFunctionType.Sigmoid
                        )
                        nc.vector.tensor_mul(y_sb[:], t3[:], sg[:])
                    nc.sync.dma_start(
                        ys_ap[rows, it * TC : (it + 1) * TC], y_sb[:]
                    )

    nc.compile()
    _PROG_CACHE[key] = nc
    return nc


def _shard_inputs(x: np.ndarray, kern: np.ndarray):
    """Full [B,T,C] inputs -> 8 per-core {xs, ks} maps (channel sharding)."""
    # xs_all rows ordered r = c*B + b; columns: [PAD zeros | x[b, :, c]]
    xs_all = np.empty((C * B, PAD + T), dtype=np.float32)
    xs_all[:, :PAD] = 0.0
    # view of the payload as [C, B, T]; strided scatter from x^T
    xs_all[:, PAD:].reshape(C, B, T)[:] = x.transpose(2, 0, 1)
    ks_all = np.repeat(kern.T, B, axis=0)  # [C*B, K], row r = c*B + b
    ks_all = np.ascontiguousarray(ks_all, dtype=np.float32)
    in_maps = [
        {
            "xs": xs_all[i * ROWS : (i + 1) * ROWS],
            "ks": ks_all[i * ROWS : (i + 1) * ROWS],
        }
        for i in range(NCORES)
    ]
    return in_maps


def kernel(x: np.ndarray, kernel: np.ndarray):
    from concourse import bass_utils

    x = np.ascontiguousarray(x, dtype=np.float32)
    kern = np.ascontiguousarray(kernel, dtype=np.float32)

    variant = os.environ.get("CONV_VARIANT", "split")
    nc = _build_program(use_silu=True, variant=variant)
    in_maps = _shard_inputs(x, kern)

    trace = os.environ.get("CONV_TRACE", "0") == "1"
    res = bass_utils.run_bass_kernel_spmd(
        nc, in_maps, list(range(NCORES)), trace=trace
    )
    globals()["LAST_RESULTS"] = res  # for the test harness (exec_time_ns etc.)

    ys_all = np.concatenate([out["ys"] for out in res.results], axis=0)
    y = np.empty((B, T, C), dtype=np.float32)
    y.transpose(2, 0, 1)[:] = ys_all.reshape(C, B, T)

    next_cache = np.ascontiguousarray(x[:, T - PAD :, :])
    return y, next_cache


# revision 8
# speedup vs baseline: 3.1671x; 3.1671x over previous
"""Trainium2 Bass kernel: causal depthwise short conv1d + SiLU.

Problem: x [B=4, T=4096, C=2048] f32, kernel [K=4, C=2048] f32.
  y[b, t, c] = silu(sum_j kernel[j, c] * x[b, t - j, c])   (zero left-pad)
  next_cache = x[:, T-K+1:, :]

Strategy:
  - Each (b, c) pair is an independent length-T sequence -> B*C = 8192 rows.
  - Shard channels across the 8 cores: core i handles channels
    [i*256, (i+1)*256) -> 1024 rows of [PAD + T] (zero pre-padded).
  - On-chip layout: partition = row, free dim = time. The 4 taps become
    per-partition tensor_scalar / scalar_tensor_tensor fused mul-adds on
    DVE/GPSIMD; SiLU runs on the scalar engine.
"""

import os
import sys

import numpy as np

_TRN_REPO = "/opt/trn_rl_repo"
if _TRN_REPO not in sys.path:
    sys.path.insert(0, _TRN_REPO)

B, T, C, K = 4, 4096, 2048, 4
PAD = K - 1
NCORES = 8
CPC = C // NCORES       # 256 channels per core
ROWS = CPC * B          # 1024 rows per core
TC = 2048               # time-chunk per tile

_PROG_CACHE: dict = {}


def _build_program(use_silu: bool = True, variant: str = "split", loop: int = 1):
    """Build the per-core Bass program (identical on all 8 cores).

    loop > 1 wraps the whole body in a hardware For_i that redoes the
    (idempotent) work `loop` times — benchmarking only.
    """
    key = (use_silu, variant, loop)
    if key in _PROG_CACHE:
        return _PROG_CACHE[key]

    import concourse.tile as tile
    from concourse import bacc, mybir

    nc = bacc.Bacc("TRN2")
    xs = nc.dram_tensor("xs", [ROWS, PAD + T], mybir.dt.float32, kind="ExternalInput")
    ks = nc.dram_tensor("ks", [ROWS, K], mybir.dt.float32, kind="ExternalInput")
    ys = nc.dram_tensor("ys", [ROWS, T], mybir.dt.float32, kind="ExternalOutput")

    MULT = mybir.AluOpType.mult
    ADD = mybir.AluOpType.add
    F32 = mybir.dt.float32

    xs_ap, ks_ap, ys_ap = xs.ap(), ks.ap(), ys.ap()

    from contextlib import ExitStack

    with tile.TileContext(nc) as tc:
        with ExitStack() as stack:
            kpool = stack.enter_context(tc.tile_pool(name="kp", bufs=2))
            xpool = stack.enter_context(tc.tile_pool(name="xp", bufs=3))
            tpool = stack.enter_context(tc.tile_pool(name="tp", bufs=3))
            ypool = stack.enter_context(tc.tile_pool(name="yp", bufs=3))
            if loop > 1:
                stack.enter_context(tc.For_i(0, loop, 1))
            for r in range(ROWS // 128):
                rows = slice(r * 128, (r + 1) * 128)
                k_sb = kpool.tile([128, K], F32)
                nc.sync.dma_start(k_sb[:], ks_ap[rows, :])
                for it in range(T // TC):
                    x_sb = xpool.tile([128, TC + PAD], F32)
                    nc.sync.dma_start(
                        x_sb[:], xs_ap[rows, it * TC : it * TC + TC + PAD]
                    )
                    if variant == "dmaonly":
                        nc.sync.dma_start(
                            ys_ap[rows, it * TC : (it + 1) * TC],
                            x_sb[:, PAD : PAD + TC],
                        )
                        continue
                    # z = sum_j k[:, j] * x[t - j]; x_sb col (PAD + t) = x[t]
                    t0 = tpool.tile([128, TC], F32, tag="t0")
                    nc.vector.tensor_scalar_mul(
                        t0[:], x_sb[:, PAD : PAD + TC], k_sb[:, 0:1]
                    )
                    t1 = tpool.tile([128, TC], F32, tag="t1")
                    nc.vector.scalar_tensor_tensor(
                        t1[:], x_sb[:, PAD - 1 : PAD - 1 + TC], k_sb[:, 1:2],
                        t0[:], MULT, ADD,
                    )
                    t2 = tpool.tile([128, TC], F32, tag="t2")
                    if variant == "split":
                        # tap 2 on GPSIMD to offload the vector engine
                        nc.gpsimd.scalar_tensor_tensor(
                            t2[:], x_sb[:, PAD - 2 : PAD - 2 + TC], k_sb[:, 2:3],
                            t1[:], MULT, ADD,
                        )
                    else:
                        nc.vector.scalar_tensor_tensor(
                            t2[:], x_sb[:, PAD - 2 : PAD - 2 + TC], k_sb[:, 2:3],
                            t1[:], MULT, ADD,
                        )
                    t3 = tpool.tile([128, TC], F32, tag="t3")
                    nc.vector.scalar_tensor_tensor(
                        t3[:], x_sb[:, PAD - 3 : PAD - 3 + TC], k_sb[:, 3:4],
                        t2[:], MULT, ADD,
                    )
                    y_sb = ypool.tile([128, TC], F32)
                    if use_silu:
                        nc.scalar.activation(
                            y_sb[:], t3[:], mybir.ActivationFunctionType.Silu
                        )
                    else:
                        # CoreSim does not implement Silu; emulate as z*sigmoid(z)
                        sg = tpool.tile([128, TC], F32, tag="sg")
                        nc.scalar.activation(
                            sg[:], t3[:], mybir.ActivationFunctionType.Sigmoid
                        )
                        nc.vector.tensor_mul(y_sb[:], t3[:], sg[:])
                    nc.sync.dma_start(
                        ys_ap[rows, it * TC : (it + 1) * TC], y_sb[:]
                    )

    nc.compile()
    _PROG_CACHE[key] = nc
    return nc


def _shard_inputs(x: np.ndarray, kern: np.ndarray):
    """Full [B,T,C] inputs -> 8 per-core {xs, ks} maps (channel sharding)."""
    # xs_all rows ordered r = c*B + b; columns: [PAD zeros | x[b, :, c]]
    xs_all = np.empty((C * B, PAD + T), dtype=np.float32)
    xs_all[:, :PAD] = 0.0
    # view of the payload as [C, B, T]; strided scatter from x^T
    xs_all[:, PAD:].reshape(C, B, T)[:] = x.transpose(2, 0, 1)
    ks_all = np.repeat(kern.T, B, axis=0)  # [C*B, K], row r = c*B + b
    ks_all = np.ascontiguousarray(ks_all, dtype=np.float32)
    in_maps = [
        {
            "xs": xs_all[i * ROWS : (i + 1) * ROWS],
            "ks": ks_all[i * ROWS : (i + 1) * ROWS],
        }
        for i in range(NCORES)
    ]
    return in_maps


def kernel(x: np.ndarray, kernel: np.ndarray):
    from concourse import bass_utils

    x = np.ascontiguousarray(x, dtype=np.float32)
    kern = np.ascontiguousarray(kernel, dtype=np.float32)

    variant = os.environ.get("CONV_VARIANT", "split")
    nc = _build_program(use_silu=True, variant=variant)
    in_maps = _shard_inputs(x, kern)

    trace = os.environ.get("CONV_TRACE", "0") == "1"
    res = bass_utils.run_bass_kernel_spmd(
        nc, in_maps, list(range(NCORES)), trace=trace
    )
    globals()["LAST_RESULTS"] = res  # for the test harness (exec_time_ns etc.)

    ys_all = np.concatenate([out["ys"] for out in res.results], axis=0)
    y = np.empty((B, T, C), dtype=np.float32)
    y.transpose(2, 0, 1)[:] = ys_all.reshape(C, B, T)

    next_cache = np.ascontiguousarray(x[:, T - PAD :, :])
    return y, next_cache
